# revision 3
# baseline (speedup 1.0000x reference)
"""Causal self-attention on 8 Trainium2 NeuronCores — v2.

Problem: x[4,2048,1024] f32; qkv = x@w_qkv+b_qkv; 16 heads x 64; causal
softmax attention; out proj w_out/b_out.

Sharding: batch(4) x head-half(2) -> 8 cores. Each core computes one batch
element and 8 heads end-to-end; host sums the two partial out-projections
per batch and adds b_out.

v2 changes vs v1:
 - x^T built by bf16-cast + DRAM round-trip + xbar transpose DMA instead of
   128 PE transposes + 128 DVE copies (frees PE, DVE and PSUM).
 - Causal masking via an accumulating PE "mask matmul" that adds
   -240*max(0, k-q) to the diagonal 128-block (exp(-30*j) == 0 after the
   0.125 softmax scale), replacing per-chunk gpsimd affine_selects that sat
   in the exp->PV dependency chain.
 - Scores: per (hp, qc, ki) one [128, 2, 512] fp32 PSUM region (2 banks)
   holding both heads of the pair; ONE exp instruction covers both -> 160
   exp instructions instead of 320, halving ACT instruction overhead.
   Regions are double-buffered so ACT runs back-to-back.
 - Normalization: po (A^T + denominator row) is copied PSUM->SBUF right
   after the last PV matmul (releasing the PSUM bank), then reciprocal +
   one partition_broadcast per head-pair + 2 DVE mults.
 - Emission order keeps PE's strict-FIFO queue stall-free: each attention
   unit emits [PV of prev batch][scores of this batch][exp of this batch]
   and interleave pacing inserts qkv/outproj matmuls between units.
"""

import sys

sys.path.insert(0, "/opt/trn_rl_repo")

import numpy as np

B, T, C = 4, 2048, 1024
H, DH = 16, 64
HPC = 8           # heads per core
DPC = HPC * DH    # 512 per-core q/k/v features
NCORES = 8

_CACHE = {}


def _build(nbody=1, skip_attn=False, skip_proj=False, xpose_mode="dma",
           exp_mode="normal"):
    """nbody > 1 replicates the whole computation, chaining y -> next x
    through DRAM scratch. Used only for device-time measurement (the
    dispatch overhead of one execution is ~0.9ms here, so a single body is
    invisible to wall-clock timing; k chained bodies make the marginal
    per-body device time measurable). skip_attn/skip_proj carve out phases
    for on-hardware attribution (output is then wrong, timing-only)."""
    import concourse.bacc as bacc
    import concourse.mybir as mybir
    import concourse.tile as tile
    from concourse.masks import make_identity

    F32 = mybir.dt.float32
    BF16 = mybir.dt.bfloat16
    Exp = mybir.ActivationFunctionType.Exp
    add_op = mybir.AluOpType.add
    mult_op = mybir.AluOpType.mult
    is_ge = mybir.AluOpType.is_ge

    nc = bacc.Bacc("TRN2", target_bir_lowering=False, debug=False,
                   num_devices=NCORES)

    xb = nc.dram_tensor("xb", [T, C], F32, kind="ExternalInput").ap()
    wq = nc.dram_tensor("wq", [C, DPC], BF16, kind="ExternalInput").ap()
    wk = nc.dram_tensor("wk", [C, DPC], BF16, kind="ExternalInput").ap()
    wv = nc.dram_tensor("wv", [C, DPC], BF16, kind="ExternalInput").ap()
    bq = nc.dram_tensor("bq", [DPC], F32, kind="ExternalInput").ap()
    bk = nc.dram_tensor("bk", [DPC], F32, kind="ExternalInput").ap()
    bv = nc.dram_tensor("bv", [DPC], F32, kind="ExternalInput").ap()
    wo = nc.dram_tensor("wo", [DPC, C], BF16, kind="ExternalInput").ap()
    y = nc.dram_tensor("y", [T, C], F32, kind="ExternalOutput").ap()

    NT = T // 128          # 16 t-tiles of 128
    NCC = C // 128         # 8 contraction chunks for qkv proj
    NDC = DPC // 128       # 4 d-chunks of per-core features
    NQC = T // 512         # 4 q-chunks of 512

    with tile.TileContext(nc) as tc:
        import contextlib
        with contextlib.ExitStack() as stk:
            singles = stk.enter_context(tc.tile_pool(name="singles", bufs=1))
            small = stk.enter_context(tc.tile_pool(name="small", bufs=3))
            ptp = stk.enter_context(tc.tile_pool(name="ptp", bufs=4))
            p1 = stk.enter_context(tc.tile_pool(name="p1", bufs=1))
            px = stk.enter_context(tc.tile_pool(name="px", bufs=2))
            pdram = stk.enter_context(
                tc.tile_pool(name="pdram", bufs=2, space="DRAM"))
            pnrm = stk.enter_context(tc.tile_pool(name="pnrm", bufs=2))
            ps_big = stk.enter_context(
                tc.tile_pool(name="ps_big", bufs=2, space="PSUM"))
            ps_w = stk.enter_context(
                tc.tile_pool(name="ps_w", bufs=2, space="PSUM"))
            ps_o = stk.enter_context(
                tc.tile_pool(name="ps_o", bufs=2, space="PSUM"))

            QT = singles.tile([128, NDC, T], BF16, tag="QT")
            KT = singles.tile([128, NDC, T], BF16, tag="KT")
            V = singles.tile([128, NT, HPC, DH + 1], BF16, tag="V")
            AT = singles.tile([128, NDC, T], BF16, tag="AT")

            bq_sb = singles.tile([128, NDC], F32, tag="bq_sb")
            bk_sb = singles.tile([128, NDC], F32, tag="bk_sb")
            bv_sb = singles.tile([1, DPC], F32, tag="bv_sb")
            bvb = singles.tile([128, DPC], F32, tag="bvb")
            nc.scalar.dma_start(
                out=bq_sb, in_=bq.rearrange("(d p) -> p d", p=128))
            nc.scalar.dma_start(
                out=bk_sb, in_=bk.rearrange("(d p) -> p d", p=128))
            nc.scalar.dma_start(
                out=bv_sb, in_=bv.rearrange("(a b) -> a b", a=1))
            nc.gpsimd.partition_broadcast(bvb, bv_sb, channels=128)

            # ones columns of V_aug
            nc.vector.memset(V[:, :, :, DH:DH + 1], 1.0)
            if skip_attn:
                nc.vector.memset(AT, 0.0)     # timing-only build
            if skip_proj:
                nc.vector.memset(QT, 0.0)
                nc.vector.memset(KT, 0.0)
                nc.vector.memset(V[:, :, :, 0:DH], 0.0)

            # causal-mask matmul operands:
            #  mask_A[d, k] = -240 if d < k else 0   (stationary)
            #  mask_B[d, q] = 1    if d >= q else 0  (moving)
            #  (mask_A.T @ mask_B)[k, q] = -240 * max(0, k - q)
            # causal triangle for the diagonal 128-blocks:
            # tri[k, q] = 1 if q >= k else 0; applied to pt by a DVE
            # multiply AFTER the exp (cheap on DVE, and the lag-2 PV gives
            # it two batches of slack), keeping mask work off the saturated
            # PE instruction stream.
            tri = singles.tile([128, 128], BF16, tag="tri")
            nc.vector.memset(tri, 1.0)
            nc.gpsimd.affine_select(
                out=tri, in_=tri, compare_op=is_ge, fill=0.0,
                base=0, pattern=[[1, 128]], channel_multiplier=-1)

            # preload the exp table off the critical path
            warm = singles.tile([1, 16], F32, tag="warm")
            nc.vector.memset(warm, 0.0)
            nc.scalar.activation(out=warm, in_=warm, func=Exp, scale=1.0)

            ident = singles.tile([128, 128], F32, tag="ident")
            make_identity(nc, ident)

            pt_const = None
            if exp_mode == "none":
                pt_const = singles.tile([128, 2, 512], BF16, tag="pt_const")
                nc.vector.memset(pt_const, 0.5)

            # weights (bf16 casts of the fp32 DMAs)
            wq_sb = p1.tile([128, NCC, DPC], BF16, tag="wq_sb")
            wk_sb = p1.tile([128, NCC, DPC], BF16, tag="wk_sb")
            wv_sb = p1.tile([128, NCC, DPC], BF16, tag="wv_sb")
            wo_sb = p1.tile([128, NDC, C], BF16, tag="wo_sb")

            def units_xload(tq, x_src):
                """x quarter: DMA in (f32) -> bf16 cast -> DRAM store ->
                xbar transpose DMA into xT_q[tq % 2]. The transpose is
                split in two so the first qkv matmuls can start on
                contraction chunks 0-3 while chunks 4-7 are still in
                flight."""
                t0 = tq * 512
                units = []
                x_bf = px.tile([128, 4, C], BF16, tag="x_bf", bufs=1)
                xbf_t = pdram.tile([512, C], BF16, tag="xbf_t")

                def load_cast(tt):
                    def f():
                        x_sb = px.tile([128, C], F32, tag="x_sb")
                        nc.sync.dma_start(
                            out=x_sb,
                            in_=x_src[t0 + tt * 128:t0 + (tt + 1) * 128, :])
                        nc.vector.tensor_copy(out=x_bf[:, tt, :], in_=x_sb)
                    return f

                def store():
                    nc.sync.dma_start(
                        out=xbf_t.rearrange("(a p) c -> p a c", p=128),
                        in_=x_bf)

                def xpose(half):
                    def f():
                        nc.sync.dma_start(
                            out=xT_q[tq % 2][:, half * 4:(half + 1) * 4, :],
                            in_=xbf_t[:, half * 512:(half + 1) * 512],
                            transpose=True)
                    return f

                for tt in range(4):
                    units.append(load_cast(tt))
                units.append(store)
                units.append(xpose(0))
                units.append(xpose(1))
                return units

            def units_xload_pe(tq, x_src):
                """Quarter 0 variant: PE transposes straight from the f32
                x tiles. At kernel start PE is otherwise idle, and this
                skips the cast->DRAM->xbar-transpose latency chain (~20us
                shorter time-to-first-qkv-matmul)."""
                t0 = tq * 512
                units = []

                def one(tt):
                    def f():
                        x_sb = px.tile([128, C], F32, tag="x_sb")
                        nc.sync.dma_start(
                            out=x_sb,
                            in_=x_src[t0 + tt * 128:t0 + (tt + 1) * 128, :])
                        for cc in range(NCC):
                            pst = ps_w.tile([128, 128], F32, tag="w")
                            nc.tensor.transpose(
                                pst, x_sb[:, cc * 128:(cc + 1) * 128],
                                ident)
                            nc.vector.tensor_copy(
                                out=xT_q[tq % 2][:, cc,
                                                 tt * 128:(tt + 1) * 128],
                                in_=pst)
                    return f

                for tt in range(4):
                    units.append(one(tt))
                return units

            xT_q = [p1.tile([128, NCC, 512], BF16, tag=f"xT_q{i}",
                            name=f"xT_q{i}") for i in range(2)]

            def units_wload():
                """Weights arrive pre-cast to bf16 from the host: straight
                DMA, no staging or DVE casts."""
                units = []

                def wload(w_dram, w_bf, cc):
                    def f():
                        # scalar (ACT) queue is the second HWDGE: weight
                        # DMAs there don't head-of-line block the x
                        # pipeline on the sync queue.
                        nc.scalar.dma_start(
                            out=w_bf[:, cc, :],
                            in_=w_dram[cc * 128:(cc + 1) * 128, :])
                    return f

                def oload(dc):
                    def f():
                        nc.scalar.dma_start(
                            out=wo_sb[:, dc, :],
                            in_=wo[dc * 128:(dc + 1) * 128, :])
                    return f

                for w_dram, w_bf in ((wq, wq_sb), (wk, wk_sb), (wv, wv_sb)):
                    for cc in range(NCC):
                        units.append(wload(w_dram, w_bf, cc))
                for dc in range(NDC):
                    units.append(oload(dc))
                return units

            def units_qkv(tq):
                """Emission units for quarter tq of qkv matmuls."""
                t0 = tq * 512
                xq = xT_q[tq % 2]
                units = []

                def qk(w_bf, OUT, b_col, dc):
                    # split into two emission halves (same PSUM accumulate
                    # group) so attention/exp work interleaves at ~0.5us
                    # granularity in the PE queue instead of ~1us
                    psq_box = {}

                    def f1():
                        psq_box["t"] = ps_w.tile([128, 512], F32, tag="w",
                                                 name="psq")
                        for cc in range(NCC // 2):
                            nc.tensor.matmul(
                                psq_box["t"],
                                w_bf[:, cc, dc * 128:(dc + 1) * 128],
                                xq[:, cc, :],
                                start=(cc == 0), stop=False)

                    def f2():
                        psq = psq_box["t"]
                        for cc in range(NCC // 2, NCC):
                            nc.tensor.matmul(
                                psq,
                                w_bf[:, cc, dc * 128:(dc + 1) * 128],
                                xq[:, cc, :],
                                start=False, stop=(cc == NCC - 1))
                        nc.vector.tensor_scalar_add(
                            out=OUT[:, dc, t0:t0 + 512], in0=psq,
                            scalar1=b_col[:, dc:dc + 1])
                    return [f1, f2]

                def vproj(tt):
                    def f():
                        psv = ps_w.tile([128, 512], F32, tag="w")
                        for cc in range(NCC):
                            nc.tensor.matmul(
                                psv,
                                xq[:, cc, tt * 128:(tt + 1) * 128],
                                wv_sb[:, cc, :],
                                start=(cc == 0), stop=(cc == NCC - 1))
                        nc.vector.tensor_tensor(
                            out=V[:, tq * 4 + tt, :, 0:DH],
                            in0=psv.rearrange("p (h c) -> p h c", h=HPC),
                            in1=bvb.rearrange("p (h c) -> p h c", h=HPC),
                            op=add_op)
                    return f

                for dc in range(NDC):
                    units.extend(qk(wq_sb, QT, bq_sb, dc))
                    units.extend(qk(wk_sb, KT, bk_sb, dc))
                for tt in range(4):
                    units.append(vproj(tt))
                return units

            def units_attention(hp, qc):
                """Emission units for one head-pair's attention q-chunk.

                Batch b == k-chunk ki covers both pars in one [128, 2, 512]
                PSUM region; unit b emits [PV of batch b-1][scores of b]
                [exp of b] so PE's FIFO never waits on ACT when pacing
                inserts filler between units."""
                q0 = qc * 512
                nkc = (qc + 1) * 4      # causal k-chunks of 128
                po = [ps_o.tile([128, 512], F32, tag="o", name=f"po{par}")
                      for par in range(2)]
                state = {"pend": []}    # [(ki, pt, off)] awaiting PV
                units = []

                def pv_emit(ki, pt, off):
                    for par in range(2):
                        h = hp * 2 + par
                        nc.tensor.matmul(
                            po[par][0:DH + 1, off:512],
                            V[:, ki, h, :],
                            pt[:, par, off:512],
                            start=(ki == 0), stop=(ki == nkc - 1),
                            skip_group_check=True)

                def batch(ki):
                    k0 = ki * 128
                    off = max(0, k0 - q0)

                    def f():
                        pss = ps_big.tile([128, 2, 512], F32, tag="s")
                        for par in range(2):
                            nc.tensor.matmul(
                                pss[:, par, off:512],
                                KT[par * 64:(par + 1) * 64, hp,
                                   k0:k0 + 128],
                                QT[par * 64:(par + 1) * 64, hp,
                                   q0 + off:q0 + 512],
                                start=True, stop=True)
                        if exp_mode == "none":
                            # timing probe: no exp at all (WRONG output)
                            state["pend"].append((ki, pt_const, off))
                            if len(state["pend"]) > 2:
                                pv_emit(*state["pend"].pop(0))
                            return
                        pt = ptp.tile([128, 2, 512], BF16, tag="pt")
                        if exp_mode == "tiny":
                            # timing probe: 1/8-size exp (WRONG output)
                            nc.scalar.activation(
                                out=pt[:, :, off:off + 64],
                                in_=pss[:, :, off:off + 64],
                                func=Exp, scale=0.125)
                        elif exp_mode == "split":
                            # timing probe: same elements, 2 instructions
                            for par in range(2):
                                nc.scalar.activation(
                                    out=pt[:, par, off:512],
                                    in_=pss[:, par, off:512],
                                    func=Exp, scale=0.125)
                        else:
                            nc.scalar.activation(
                                out=pt[:, :, off:512],
                                in_=pss[:, :, off:512],
                                func=Exp, scale=0.125)
                        if k0 >= q0:
                            # zero the upper-triangle of the diagonal block
                            for par in range(2):
                                nc.vector.tensor_tensor(
                                    out=pt[:, par, off:off + 128],
                                    in0=pt[:, par, off:off + 128],
                                    in1=tri, op=mult_op)
                        # PV lags TWO batches: when PE reaches a PV matmul
                        # its exp has long finished, so PE's strict FIFO
                        # never blocks on ACT and ACT streams exps
                        # back-to-back.
                        state["pend"].append((ki, pt, off))
                        if len(state["pend"]) > 2:
                            pv_emit(*state["pend"].pop(0))
                    return f

                def finish():
                    # trailing PVs, then free po into SBUF + normalize.
                    # A^T rows land on the same partitions they'll occupy in
                    # AT (par0 -> 0:64, par1 -> 64:128, via DVE partition
                    # shift) because the gpsimd norm-multiply is
                    # partition-local and cannot shift.
                    for args in state["pend"]:
                        pv_emit(*args)
                    state["pend"] = []
                    asb = pnrm.tile([128, 512], F32, tag="asb")
                    rec2 = pnrm.tile([1, 1024], F32, tag="rec2")
                    for par in range(2):
                        nc.vector.tensor_copy(
                            out=asb[par * 64:(par + 1) * 64, :],
                            in_=po[par][0:DH, :])
                        nc.vector.reciprocal(
                            out=rec2[0:1, par * 512:(par + 1) * 512],
                            in_=po[par][DH:DH + 1, :])
                    bc = pnrm.tile([128, 1024], F32, tag="bc")
                    nc.gpsimd.partition_broadcast(bc, rec2, channels=128)
                    for par in range(2):
                        # gpsimd (idle engine) does the multiply; all three
                        # operands sit on the same partitions.
                        nc.gpsimd.tensor_tensor(
                            out=AT[par * 64:(par + 1) * 64, hp,
                                   q0:q0 + 512],
                            in0=asb[par * 64:(par + 1) * 64, :],
                            in1=bc[par * 64:(par + 1) * 64,
                                   par * 512:(par + 1) * 512],
                            op=mult_op)

                for ki in range(nkc):
                    units.append(batch(ki))
                units.append(finish)
                return units

            def units_outproj(tt, y_dst):
                def one(cc2):
                    def f():
                        py = ps_w.tile([128, 512], F32, tag="w")
                        for hp in range(NDC):
                            nc.tensor.matmul(
                                py,
                                AT[:, hp, tt * 128:(tt + 1) * 128],
                                wo_sb[:, hp, cc2 * 512:(cc2 + 1) * 512],
                                start=(hp == 0), stop=(hp == NDC - 1))
                        ysb = small.tile([128, 512], F32, tag="ysb", bufs=3)
                        nc.vector.tensor_copy(out=ysb, in_=py)
                        nc.sync.dma_start(
                            out=y_dst[tt * 128:(tt + 1) * 128,
                                      cc2 * 512:(cc2 + 1) * 512],
                            in_=ysb)
                    return f
                return [one(0), one(1)]

            def interleave_emit(a_units, b_units):
                """Emit a_units (PE-heavy fillers) and b_units (ACT-gated
                attention) round-robin, pacing a to spread across b."""
                na, nb = len(a_units), len(b_units)
                ai = 0
                for i, u in enumerate(b_units):
                    u()
                    target = (i + 1) * na // nb
                    while ai < target:
                        a_units[ai]()
                        ai += 1
                while ai < na:
                    a_units[ai]()
                    ai += 1

            def emit_body(x_src, y_dst, pre_units=()):
                # body-0's x pipeline is emitted FIRST so its DMAs lead the
                # shared DMA pipe; the weight loads (pre_units) follow and
                # fill the transpose latency. Quarter 0 transposes on PE.
                if not skip_proj:
                    for u in units_xload_pe(0, x_src):
                        u()
                for u in pre_units:
                    u()
                for tq in range(NQC):
                    a_units = []
                    if not skip_proj:
                        a_units += units_qkv(tq)
                        if tq + 1 < NQC:
                            a_units += units_xload(tq + 1, x_src)
                    b_units = []
                    if tq >= 1 and not skip_attn:
                        for hp in range(NDC):
                            b_units += units_attention(hp, tq - 1)
                    if b_units:
                        interleave_emit(a_units, b_units)
                    else:
                        for u in a_units:
                            u()
                # tail: quarter-3 attention (ACT-bound) + remaining
                # out-proj as PE filler
                tail_attn = []
                if not skip_attn:
                    for hp in range(NDC):
                        tail_attn += units_attention(hp, NQC - 1)
                tail_proj = []
                if not skip_proj:
                    for tt in range(0, (NQC - 1) * 4):
                        tail_proj += units_outproj(tt, y_dst)
                interleave_emit(tail_proj, tail_attn)
                if not skip_proj:
                    for tt in range((NQC - 1) * 4, NQC * 4):
                        for u in units_outproj(tt, y_dst):
                            u()
                elif nbody > 1:
                    # keep the y -> next-x chain alive for timing builds
                    dummy = small.tile([128, C], F32, tag="ysb")
                    nc.vector.memset(dummy, 0.0)
                    nc.sync.dma_start(out=y_dst[0:128, :], in_=dummy)

            # nbody chained bodies (nbody == 1 in the graded kernel; > 1
            # only for device-time measurement)
            chain = [None] * (nbody - 1)
            for i in range(nbody - 1):
                chain[i] = pdram.tile([T, C], F32, tag="ychain",
                                      name=f"ychain{i}", bufs=2)
            for i in range(nbody):
                x_src = xb if i == 0 else chain[i - 1]
                y_dst = y if i == nbody - 1 else chain[i]
                emit_body(x_src, y_dst,
                          pre_units=units_wload() if i == 0 else ())

    nc.compile()
    return nc


LAST_RESULTS = None


def make_in_maps(x, w_qkv, b_qkv, w_out):
    import ml_dtypes
    BF = ml_dtypes.bfloat16
    in_maps = []
    for core in range(NCORES):
        b = core // 2
        h0 = (core % 2) * HPC
        d0 = h0 * DH
        in_maps.append({
            "xb": np.ascontiguousarray(x[b]),
            "wq": np.ascontiguousarray(w_qkv[:, d0:d0 + DPC]).astype(BF),
            "wk": np.ascontiguousarray(
                w_qkv[:, C + d0:C + d0 + DPC]).astype(BF),
            "wv": np.ascontiguousarray(
                w_qkv[:, 2 * C + d0:2 * C + d0 + DPC]).astype(BF),
            "bq": np.ascontiguousarray(b_qkv[d0:d0 + DPC]),
            "bk": np.ascontiguousarray(b_qkv[C + d0:C + d0 + DPC]),
            "bv": np.ascontiguousarray(b_qkv[2 * C + d0:2 * C + d0 + DPC]),
            "wo": np.ascontiguousarray(w_out[d0:d0 + DPC, :]).astype(BF),
        })
    return in_maps


def kernel(x, w_qkv, b_qkv, w_out, b_out):
    global LAST_RESULTS
    from concourse import bass_utils

    x = np.ascontiguousarray(np.asarray(x, dtype=np.float32))
    w_qkv = np.ascontiguousarray(np.asarray(w_qkv, dtype=np.float32))
    b_qkv = np.ascontiguousarray(np.asarray(b_qkv, dtype=np.float32))
    w_out = np.ascontiguousarray(np.asarray(w_out, dtype=np.float32))
    b_out = np.ascontiguousarray(np.asarray(b_out, dtype=np.float32))

    if "nc" not in _CACHE:
        _CACHE["nc"] = _build()
    nc = _CACHE["nc"]

    in_maps = make_in_maps(x, w_qkv, b_qkv, w_out)

    res = bass_utils.run_bass_kernel_spmd(
        nc, in_maps, core_ids=list(range(NCORES)))
    LAST_RESULTS = res

    out = np.empty((B, T, C), dtype=np.float32)
    for b in range(B):
        out[b] = res.results[2 * b]["y"] + res.results[2 * b + 1]["y"] + b_out
    return out
